# revision 1
# baseline (speedup 1.0000x reference)
"""DeepSeek-V3 style MoE gate (nn_Gate) for Trainium2, 8-core data-parallel.

fp16-main + fp8-DoubleRow-correction scheme (exact to ~2^-15):
  logits = wa16 . xa16            (fp16 x fp16: 11-bit operands, exact)
         + [w8 . xl8 + wb8 . xa8] / 4096   (fp8 DoubleRow, both scaled 2^12)
where xa16 = fp16(x), xl8 = fp8((x - xa16)*4096), xa8 = fp8(xa16),
      wa16 = fp16(w), wb8 = fp8((w - wa16)*4096), w8 = fp8(w).
x-side planes stream from DRAM (4B/elem total, split across the SP and
ACT hardware DMA queues); w-side planes are SBUF-resident.
PSUM bank A accumulates the fp16 pass, bank B both DoubleRow terms; a
fused DVE op combines them, ACT applies sigmoid, PE transposes to
token-major, and the DVE runs the group-limited top-8 selection.
"""
import numpy as np
import ml_dtypes
import concourse.bass as bass
import concourse.tile as tile
from concourse import bacc, mybir
from concourse.masks import make_identity
from concourse.bass_utils import run_bass_kernel_spmd

AOT = mybir.AluOpType
F32 = mybir.dt.float32
F16 = mybir.dt.float16
FP8 = mybir.dt.float8e4
DR = mybir.MatmulPerfMode.DoubleRow

N_TOKENS = 8192
K = 7168
NK = K // 128
NKK = K // 256
E = 256
N_CORES = 8
T_CORE = N_TOKENS // N_CORES
TB = 512
NTB = T_CORE // TB
S = 4096.0


def _topk_chain(nc, pool, scores, bias_b, wout, iout):
    """Group-limited top-8 for one 128-token tile.

    scores: [128,256] f32 SBUF sigmoid scores; bias_b: [128,256] f32
    broadcast bias; wout/iout: DRAM APs [128,8] f32/int32.
    """
    u = pool.tile([128, 256], F32, tag="u")
    nc.vector.tensor_add(u[:], scores[:], bias_b[:])
    u3 = u[:].rearrange("p (g e) -> p g e", g=8)
    # top-2 per group of 32: group max, zap it, group max again
    gmax1 = pool.tile([128, 8], F32, tag="gmax1")
    nc.vector.tensor_reduce(gmax1[:], u3, axis=mybir.AxisListType.X, op=AOT.max)
    u_z = pool.tile([128, 256], F32, tag="u_z")
    nc.vector.match_replace(u_z[:], gmax1[:], u[:], -1e30)
    gmax2 = pool.tile([128, 8], F32, tag="gmax2")
    nc.vector.tensor_reduce(gmax2[:], u_z[:].rearrange("p (g e) -> p g e", g=8),
                            axis=mybir.AxisListType.X, op=AOT.max)
    g2sum = pool.tile([128, 8], F32, tag="g2sum")
    nc.vector.tensor_add(g2sum[:], gmax1[:], gmax2[:])
    # top-4 groups: threshold at 4th largest group score
    gtop = pool.tile([128, 8], F32, tag="gtop")
    nc.vector.max(out=gtop[:], in_=g2sum[:])
    gmask = pool.tile([128, 8], F32, tag="gmask")
    nc.vector.tensor_scalar(gmask[:], g2sum[:], gtop[:, 3:4], None, op0=AOT.is_ge)
    # mask = multiply by 0/1 exactly like the reference
    u_m = pool.tile([128, 256], F32, tag="u_m")
    nc.vector.tensor_tensor(
        out=u_m[:].rearrange("p (g e) -> p g e", g=8),
        in0=u3,
        in1=gmask[:].unsqueeze(-1).to_broadcast([128, 8, 32]),
        op=AOT.mult,
    )
    # global top-8 of masked biased scores
    fvals = pool.tile([128, 8], F32, tag="fvals")
    nc.vector.max(out=fvals[:], in_=u_m[:])
    fidx = pool.tile([128, 8], mybir.dt.uint32, tag="fidx")
    nc.vector.max_index(fidx[:], fvals[:], u_m[:])
    # mark selected positions, pull original sigmoid scores there
    u2 = pool.tile([128, 256], F32, tag="u2")
    nc.vector.match_replace(u2[:], fvals[:], u_m[:], 1e38)
    sel01 = pool.tile([128, 256], F32, tag="sel01")
    nc.vector.tensor_scalar(sel01[:], u2[:], 1e30, None, op0=AOT.is_ge)
    wsel = pool.tile([128, 256], F32, tag="wsel")
    nc.vector.tensor_mul(wsel[:], scores[:], sel01[:])
    wvals = pool.tile([128, 8], F32, tag="wvals")
    nc.vector.max(out=wvals[:], in_=wsel[:])
    widx = pool.tile([128, 8], mybir.dt.uint32, tag="widx")
    nc.vector.max_index(widx[:], wvals[:], wsel[:])
    # align score-ordered (wvals, widx) pairs to the biased order fidx
    fidx_f = pool.tile([128, 8], F32, tag="fidx_f")
    nc.vector.tensor_copy(fidx_f[:], fidx[:])
    widx_f = pool.tile([128, 8], F32, tag="widx_f")
    nc.vector.tensor_copy(widx_f[:], widx[:])
    eq = pool.tile([128, 64], F32, tag="eq")
    nc.vector.tensor_tensor(
        out=eq[:].rearrange("p (a b) -> p a b", a=8),
        in0=fidx_f[:].unsqueeze(-1).to_broadcast([128, 8, 8]),
        in1=widx_f[:].unsqueeze(1).to_broadcast([128, 8, 8]),
        op=AOT.is_equal,
    )
    wa = pool.tile([128, 64], F32, tag="wa")
    nc.vector.tensor_tensor(
        out=wa[:].rearrange("p (a b) -> p a b", a=8),
        in0=eq[:].rearrange("p (a b) -> p a b", a=8),
        in1=wvals[:].unsqueeze(1).to_broadcast([128, 8, 8]),
        op=AOT.mult,
    )
    w_al = pool.tile([128, 8], F32, tag="w_al")
    nc.vector.tensor_reduce(w_al[:], wa[:].rearrange("p (a b) -> p a b", a=8),
                            axis=mybir.AxisListType.X, op=AOT.add)
    # renormalize and scale by 2.5
    denom = pool.tile([128, 1], F32, tag="denom")
    nc.vector.tensor_reduce(denom[:], w_al[:], axis=mybir.AxisListType.X, op=AOT.add)
    recip = pool.tile([128, 1], F32, tag="recip")
    nc.vector.reciprocal(recip[:], denom[:])
    wfin = pool.tile([128, 8], F32, tag="wfin")
    nc.vector.tensor_scalar(wfin[:], w_al[:], recip[:, 0:1], 2.5, op0=AOT.mult, op1=AOT.mult)
    nc.gpsimd.dma_start(wout, wfin[:])
    nc.gpsimd.dma_start(iout, fidx[:].bitcast(mybir.dt.int32))


def build_kernel(reps=None):
    nc = bacc.Bacc("TRN2", target_bir_lowering=False, debug=False,
                   enable_asserts=False, num_devices=N_CORES)
    # host-prepacked tiles: row block (tb*NKK+kk)*128 holds a [128, ...]
    # SBUF-layout tile with >=2KB contiguous DRAM rows for DMA efficiency.
    # xa_p[.., i*512+t] = fp16 x^T[kk*256+i*128+p, tb*512+t]
    # x8_p[.., 0:1024] = xl8 pair, x8_p[.., 1024:2048] = xa8 pair (same layout)
    xa_in = nc.dram_tensor("xa_p", [NTB * NKK * 128, 2 * TB], F16, kind="ExternalInput").ap()
    x8_in = nc.dram_tensor("x8_p", [NTB * NKK * 128, 4 * TB], FP8, kind="ExternalInput").ap()
    wa_in = nc.dram_tensor("wa", [K, E], F16, kind="ExternalInput").ap()
    w8_in = nc.dram_tensor("w8", [K, E], FP8, kind="ExternalInput").ap()
    wb8_in = nc.dram_tensor("wb8", [K, E], FP8, kind="ExternalInput").ap()
    bias_in = nc.dram_tensor("biasb", [128, E], F32, kind="ExternalInput").ap()
    wout = nc.dram_tensor("wout", [T_CORE, 8], F32, kind="ExternalOutput").ap()
    iout = nc.dram_tensor("iout", [T_CORE, 8], mybir.dt.int32, kind="ExternalOutput").ap()

    import contextlib
    with tile.TileContext(nc) as tc:
        with (
            tc.tile_pool(name="wres", bufs=1) as wres,
            tc.tile_pool(name="consts", bufs=1) as consts,
            tc.tile_pool(name="xs", bufs=8) as xs,
            tc.tile_pool(name="x8s", bufs=8) as x8s,
            tc.tile_pool(name="mmps", bufs=3, space="PSUM") as mmps,
            tc.tile_pool(name="tps", bufs=2, space="PSUM") as tps,
            tc.tile_pool(name="sig", bufs=4) as sigp,
            tc.tile_pool(name="sc", bufs=8) as scp,
            tc.tile_pool(name="chain", bufs=3) as chain,
        ):
            wa_t = wres.tile([128, NK * E], F16, tag="wa_t")
            nc.sync.dma_start(
                wa_t[:].rearrange("p (nk e) -> p nk e", e=E),
                wa_in.rearrange("(nk p) e -> p nk e", p=128))
            w8_t = wres.tile([128, NKK * 2 * E], FP8, tag="w8_t")
            nc.sync.dma_start(
                w8_t[:].rearrange("p (nkk two e) -> p nkk two e", two=2, e=E),
                w8_in.rearrange("(nkk two p) e -> p nkk two e", two=2, p=128))
            wb8_t = wres.tile([128, NKK * 2 * E], FP8, tag="wb8_t")
            nc.sync.dma_start(
                wb8_t[:].rearrange("p (nkk two e) -> p nkk two e", two=2, e=E),
                wb8_in.rearrange("(nkk two p) e -> p nkk two e", two=2, p=128))
            bias_b = consts.tile([128, E], F32, tag="bias_b")
            nc.sync.dma_start(bias_b[:], bias_in[:])
            ident = consts.tile([128, 128], F32, tag="ident")
            make_identity(nc, ident[:])

            def epilogue(lg, tb):
                # PE transposes the combined LOGITS (waits only on the DVE
                # add, not the sigmoid), then ACT applies sigmoid while
                # evacuating PSUM - sigmoid(transpose(x)) == transpose(
                # sigmoid(x)) elementwise, and this fuses away the separate
                # scores-copy. (Deferring this past the next TB's matmuls
                # measured WORSE, so it runs inline per TB.)
                for col in range(TB // 128):
                    tt = tb * (TB // 128) + col
                    scores = scp.tile([128, E], F32, tag="scores")
                    for eh in range(2):
                        tp = tps.tile([128, 128], F32, tag="tp")
                        nc.tensor.transpose(tp[:], lg[eh][:, col*128:(col+1)*128], ident[:])
                        nc.scalar.activation(scores[:, eh*128:(eh+1)*128], tp[:],
                                             mybir.ActivationFunctionType.Sigmoid)
                    _topk_chain(nc, chain, scores, bias_b,
                                wout[tt*128:(tt+1)*128, :], iout[tt*128:(tt+1)*128, :])

            loop_ctx = tc.For_i(0, reps, 1) if reps else contextlib.nullcontext()
            with loop_ctx:
                for tb in range(NTB):
                    tsl = slice(tb * TB, (tb + 1) * TB)
                    psA = [mmps.tile([128, TB], F32, tag="psA", name=f"psA_{tb}_{i}")
                           for i in range(2)]
                    psB = [mmps.tile([128, TB], F32, tag="psB", name=f"psB_{tb}_{i}")
                           for i in range(2)]
                    # interleaved fp16 main + fp8 DoubleRow corrections:
                    # keeps the SP queue (xa) and ACT queue (x8) draining
                    # concurrently, and mixes short-load fp16 matmuls between
                    # long-load DR matmuls on the PE. One prepacked tile per
                    # k-pair: xa [128,1024] f16, x8 [128,2048] fp8 (xl8|xa8).
                    for kk in range(NKK):
                        base = (tb * NKK + kk) * 128
                        xa_k = xs.tile([128, 2 * TB], F16, tag="xa_k")
                        nc.sync.dma_start(xa_k[:], xa_in[base:base+128, :])
                        x8_k = x8s.tile([128, 4 * TB], FP8, tag="x8_k")
                        nc.scalar.dma_start(x8_k[:], x8_in[base:base+128, :])
                        for i in range(2):
                            k = 2 * kk + i
                            for eh in range(2):
                                wk = wa_t[:].rearrange("p (nk e) -> p nk e", e=E)[:, k, eh*128:(eh+1)*128]
                                nc.tensor.matmul(psA[eh][:], wk,
                                                 xa_k[:, i*TB:(i+1)*TB],
                                                 start=(k == 0), stop=(k == NK - 1))
                            if i == 0:
                                for eh in range(2):
                                    w8_k = w8_t[:].rearrange("p (nkk two e) -> p nkk two e", two=2, e=E)[:, kk, :, eh*128:(eh+1)*128]
                                    nc.tensor.matmul(psB[eh][:], w8_k,
                                                     x8_k[:, 0:2*TB].rearrange("p (two t) -> p two t", two=2),
                                                     start=(kk == 0), stop=False, perf_mode=DR,
                                                     skip_group_check=True)
                            else:
                                for eh in range(2):
                                    wb8_k = wb8_t[:].rearrange("p (nkk two e) -> p nkk two e", two=2, e=E)[:, kk, :, eh*128:(eh+1)*128]
                                    nc.tensor.matmul(psB[eh][:], wb8_k,
                                                     x8_k[:, 2*TB:4*TB].rearrange("p (two t) -> p two t", two=2),
                                                     start=False, stop=(kk == NKK - 1), perf_mode=DR,
                                                     skip_group_check=True)

                    # combine on the DVE (one PSUM operand per instruction);
                    # sigmoid is applied later, after the PE transpose.
                    lg = [None, None]
                    for eh in range(2):
                        corr = sigp.tile([128, TB], F32, tag="corr", name=f"corr_{tb}_{eh}")
                        nc.vector.tensor_scalar(corr[:], psB[eh][:], 1.0 / S, None,
                                                op0=AOT.mult)
                        lg[eh] = sigp.tile([128, TB], F32, tag="lg", name=f"lg_{tb}_{eh}")
                        nc.vector.tensor_add(lg[eh][:], corr[:], psA[eh][:])

                    epilogue(lg, tb)
    nc.compile()
    return nc


def host_prep(x, weight, bias):
    x = np.ascontiguousarray(np.asarray(x, dtype=np.float32))
    weight = np.ascontiguousarray(np.asarray(weight, dtype=np.float32))
    bias = np.asarray(bias, dtype=np.float32)
    f16 = np.float16
    f8 = ml_dtypes.float8_e4m3

    wa = weight.astype(f16)
    wb8 = ((weight - wa.astype(np.float32)) * S).astype(f8)
    w8 = weight.astype(f8)
    waT = np.ascontiguousarray(wa.T)
    w8T = np.ascontiguousarray(w8.T)
    wb8T = np.ascontiguousarray(wb8.T)
    biasb = np.ascontiguousarray(np.broadcast_to(bias, (128, E)))

    xa_all = x.astype(f16)
    xl8_all = ((x - xa_all.astype(np.float32)) * S).astype(f8)
    xa8_all = xa_all.astype(f8)

    def pack(aT):
        # aT [K, T_CORE] -> [(tb kk p), (i t)] with row = value a[kk*256+i*128+p,
        # tb*512+t]; gives 128-row tiles whose DRAM rows are contiguous.
        a5 = aT.reshape(NKK, 2, 128, NTB, TB)
        return np.ascontiguousarray(
            a5.transpose(3, 0, 2, 1, 4).reshape(NTB * NKK * 128, 2 * TB))

    in_maps = []
    for c in range(N_CORES):
        sl = slice(c * T_CORE, (c + 1) * T_CORE)
        xa_p = pack(np.ascontiguousarray(xa_all[sl].T))
        xl8_p = pack(np.ascontiguousarray(xl8_all[sl].T))
        xa8_p = pack(np.ascontiguousarray(xa8_all[sl].T))
        in_maps.append({
            "xa_p": xa_p,
            "x8_p": np.ascontiguousarray(np.concatenate([xl8_p, xa8_p], axis=1)),
            "wa": waT,
            "w8": w8T,
            "wb8": wb8T,
            "biasb": biasb,
        })
    return in_maps


_CACHED = {}


def kernel(x, token_mask, weight, bias):
    in_maps = host_prep(x, weight, bias)
    if "nc" not in _CACHED:
        _CACHED["nc"] = build_kernel()
    nc = _CACHED["nc"]
    res = run_bass_kernel_spmd(nc, in_maps, core_ids=list(range(N_CORES)))
    weights_full = np.concatenate([r["wout"] for r in res.results], axis=0)
    idx_full = np.concatenate([r["iout"] for r in res.results], axis=0)
    return weights_full.astype(np.float32), idx_full.astype(np.int32)

